# revision 17
# baseline (speedup 1.0000x reference)
"""Trainium2 Bass kernel for DenseLanguageGuidanceModule.

Math (per batch b):
    fk_l = fl @ W_lk + b_lk            [77, 512]
    fv-side projections are folded away algebraically:
      a_raw = (fk_l @ W_vk^T) @ fv^T + c 1^T   (/= sqrt(512)),
        c = fk_l @ b_vk  (the b_lk@b_vk scalar cancels in both softmaxes)
      fa_v  = diag(1/s1) (E @ fv) @ W_vv + b_vv,  E = exp(a_raw/sqrt(512) + c/sqrt(512))
      fm    = diag(1/s2) E^T @ (fv_l @ fa_v^T)
      out   = fm @ W_m + b_m
    where s1 = row sums of E, s2 = column sums of E.

Distribution: pure data-parallel over batch B=32 across 8 NeuronCores
(4 batches per core), weights replicated. No collectives.

All data tiles are bfloat16 (matmuls run 1 cycle/row at any free size and
input DMA bytes are halved; the kernel is DMA-bound so this matters most).
PSUM accumulation and all reductions/scales stay fp32; the output is fp32.

Software pipelining: finals (out-tile matmul + store) of batch b are split
into 8 single-tile chunks and popped into every dependency-stall slot of
batch b+1's chain, keeping PE and the DMA engines continuously busy.
"""
import sys

sys.path.insert(0, "/opt/trn_rl_repo")

import ml_dtypes
import numpy as np

import concourse.bass as bass  # noqa: E402
import concourse.tile as tile  # noqa: E402
from concourse import bacc, mybir  # noqa: E402
from concourse.bass_utils import run_bass_kernel_spmd  # noqa: E402

P = 128
NCORES = 8
B = 32
BL = B // NCORES          # 4 batches per core
NV, DV = 1024, 768        # vision tokens / dim
NL, DL = 77, 512          # language tokens / dim
D = 512                   # shared feature dim
OD = 768                  # output dim
NLB = NL * BL             # 308: l-dim stacked across local batches

BF = mybir.dt.bfloat16
F32 = mybir.dt.float32
ISQD = 1.0 / float(np.sqrt(np.float32(D)))

AF = mybir.ActivationFunctionType


def _build():
    nc = bacc.Bacc("TRN2", target_bir_lowering=False)

    fv_d = nc.dram_tensor("fv", [BL, NV, DV], BF, kind="ExternalInput")
    fl_d = nc.dram_tensor("fl", [BL, NL, DL], BF, kind="ExternalInput")
    wkc_d = nc.dram_tensor("wkc", [DL, DV], BF, kind="ExternalInput")
    wvc_d = nc.dram_tensor("wvc", [DL, DV], BF, kind="ExternalInput")
    wme_d = nc.dram_tensor("wme", [97, OD], BF, kind="ExternalInput")
    bmr_d = nc.dram_tensor("bmr", [1, OD], BF, kind="ExternalInput")
    cpkb_d = nc.dram_tensor("cpkb", [P, 10], BF, kind="ExternalInput")
    cpkf_d = nc.dram_tensor("cpkf", [P, 12], F32, kind="ExternalInput")
    iden_d = nc.dram_tensor("iden", [P, P], BF, kind="ExternalInput")
    out_d = nc.dram_tensor("out", [BL, NV, OD], BF, kind="ExternalOutput")

    with tile.TileContext(nc) as tc:
        with (
            tc.tile_pool(name="consts", bufs=1) as cp,
            tc.tile_pool(name="lph", bufs=1) as lp,
            tc.tile_pool(name="fvn", bufs=2) as fvnp,
            tc.tile_pool(name="fvt", bufs=3) as fvtp,
            tc.tile_pool(name="eb", bufs=2) as ebp,
            tc.tile_pool(name="sm", bufs=2) as smp,
            tc.tile_pool(name="outp", bufs=4) as outp,
            tc.tile_pool(name="tp", bufs=4, space="PSUM") as tp,       # 1-bank slots
            tc.tile_pool(name="acc", bufs=2, space="PSUM") as accp,    # 2-bank slots
        ):
            # ------- small constants first: keep them off the DMA critical path
            iden = cp.tile([P, P], BF)
            nc.sync.dma_start(iden, iden_d[:, :])
            onesc = cp.tile([P, 1], BF)
            nc.vector.memset(onesc, 1.0)
            onesr = cp.tile([1, NLB], BF)
            nc.vector.memset(onesr, 1.0)

            lph_tmp = tc.tile_pool(name="lphtmp", bufs=1)
            lpt = lph_tmp.__enter__()
            # fl natural layout: [128, 3, 512] (row-chunks of 128/128/52)
            FLn = lpt.tile([P, 3, DL], BF)
            fl_flat = fl_d.rearrange("b l d -> (b l) d")
            nc.sync.dma_start(FLn[:, 0, :], fl_flat[:P])
            nc.sync.dma_start(FLn[:, 1, :], fl_flat[P : 2 * P])
            nc.sync.dma_start(FLn[: NLB - 2 * P, 2, :], fl_flat[2 * P :, :])

            # fv0 first half ahead of wkc: its transposes are the first PE work
            # after FLT; wkc ahead of the packed small consts (GT needs it next)
            FVn0 = fvnp.tile([P, 8, DV], BF, tag="fvn")
            fvb0 = fv_d[0].rearrange("(t p) d -> p t d", p=P)
            nc.sync.dma_start(FVn0[:, 0:4, :], fvb0[:, 0:4, :])
            Wkc = lpt.tile([P, 4, DV], BF)
            nc.sync.dma_start(Wkc, wkc_d.rearrange("(ko p) m -> p ko m", p=P))
            nc.sync.dma_start(FVn0[:, 4:8, :], fvb0[:, 4:8, :])

            cpkb = cp.tile([P, 10], BF)
            nc.sync.dma_start(cpkb, cpkb_d[:, :])
            cpkf = cp.tile([P, 12], F32)
            nc.sync.dma_start(cpkf, cpkf_d[:, :])
            wct = cpkb[:, 0:4]
            wcvt = cpkb[:, 4:8]
            ccvt = cpkb[:1, 8:9]
            c2t = cpkf[:, 0:6]
            c2vt = cpkf[:, 6:12]
            WmE = cp.tile([97, OD], BF)
            nc.sync.dma_start(WmE, wme_d[:, :])
            bmr = cp.tile([1, OD], BF)
            nc.sync.dma_start(bmr, bmr_d[:, :])
            Wvc = lpt.tile([P, 4, DV], BF)
            nc.sync.dma_start(Wvc, wvc_d.rearrange("(ko p) m -> p ko m", p=P))

            # ---------------- language phase part 1: FLT, GT, c ----------------
            # FLT = fl_all^T  [512(D1 on p), 308]
            FLT = lpt.tile([P, 4, NLB], BF)
            for ko in range(4):
                ps = tp.tile([P, 384], BF, tag="tp")
                for i in range(3):
                    nc.tensor.transpose(
                        ps[:, i * P : (i + 1) * P],
                        FLn[:, i, ko * P : (ko + 1) * P],
                        iden,
                    )
                if ko % 2 == 0:
                    nc.vector.tensor_copy(FLT[:, ko, :], ps[:, :NLB])
                else:
                    nc.scalar.activation(FLT[:, ko, :], ps[:, :NLB], AF.Copy)

            def _emit_tg(FVn, tg):
                fvth = fvtp.tile([P, 6, 512], BF, tag="fvt")
                for ko in range(6):
                    ps = tp.tile([P, 512], BF, tag="tp")
                    for tt in range(4):
                        t = tg * 4 + tt
                        nc.tensor.transpose(
                            ps[:, tt * P : (tt + 1) * P],
                            FVn[:, t, ko * P : (ko + 1) * P],
                            iden,
                        )
                    if (ko + tg) % 2 == 0:
                        nc.vector.tensor_copy(fvth[:, ko, :], ps)
                    else:
                        nc.scalar.activation(fvth[:, ko, :], ps, AF.Copy)
                return fvth

            # fv0's first transpose group before GT: fv0-q0 lands before wkc
            tg00 = _emit_tg(FVn0, 0)

            # GT = g^T = (W_lk @ W_vk^T)^T @ fl^T + c2 : [768, 308]
            GT = lp.tile([P, 6, NLB], BF)
            for mv in range(6):
                ps = tp.tile([P, NLB], F32, tag="tp")
                for ko in range(4):
                    nc.tensor.matmul(
                        ps, Wkc[:, ko, mv * P : (mv + 1) * P], FLT[:, ko, :],
                        start=(ko == 0), stop=(ko == 3),
                    )
                if mv % 2 == 0:
                    nc.vector.tensor_scalar_add(GT[:, mv, :], ps, c2t[:, mv, None])
                else:
                    nc.scalar.activation(
                        GT[:, mv, :], ps, AF.Identity, bias=c2t[:, mv, None]
                    )

            # cbias[:, b] = (fk_l @ b_vk) * ISQD as a column per batch : [77, 4]
            psc = tp.tile([NL, 4], F32, tag="tp")
            for b in range(BL):
                ls = b * NL
                for ko in range(4):
                    nc.tensor.matmul(
                        psc[:, b : b + 1], FLT[:, ko, ls : ls + NL],
                        wct[:, ko : ko + 1],
                        start=(ko == 0), stop=(ko == 3),
                    )
            cbias = cp.tile([NL, 4], F32)
            nc.scalar.mul(cbias, psc, ISQD)

            # persistent Y ping-pong pair: Y = m_small @ W_m  [77, 768] per
            # batch, extended with row 77 = b_m (written once; rows 0..76 are
            # rewritten every batch). The finals contract E_ext^T @ Y_ext.
            YS = []
            for _i in range(2):
                _y = lp.tile([97, OD], BF, tag=f"YS{_i}")
                nc.gpsimd.memset(_y[64:96, :], 0.0)
                nc.vector.tensor_copy(_y[96:97, :], bmr)
                YS.append(_y)

            # ---------------- pipelined per-batch machinery ----------------
            finals_q = []

            def _pop(n=1):
                for _ in range(n):
                    if finals_q:
                        finals_q.pop(0)()

            def _issue_load(nb):
                FVn = fvnp.tile([P, 8, DV], BF, tag="fvn")
                fvb = fv_d[nb].rearrange("(t p) d -> p t d", p=P)
                for q in range(2):
                    nc.sync.dma_start(
                        FVn[:, 4 * q : 4 * q + 4, :], fvb[:, 4 * q : 4 * q + 4, :]
                    )
                return FVn

            def _emit_araw_E(nb, FVTh):
                nls = nb * NL
                # a_raw = g @ fv^T -> 2x 1-bank psum [77, 512] halves
                E = ebp.tile([P, NV], BF, tag="E")
                if nb < 2:
                    # rows 77..95 are read (x0) by the finals' [0:97) lhsT;
                    # zero them once per ring buffer (never written again)
                    nc.gpsimd.memset(E[64:96, :], 0.0)
                s1p = smp.tile([NL, 2], F32, tag="s1p")
                for nv in range(2):
                    sl = tp.tile([NL, 512], F32, tag="tp")
                    for ko in range(6):
                        nc.tensor.matmul(
                            sl, GT[:, ko, nls : nls + NL],
                            FVTh[nv][:, ko, :],
                            start=(ko == 0), stop=(ko == 5),
                        )
                    nc.scalar.activation(
                        E[:NL, nv * 512 : (nv + 1) * 512], sl,
                        AF.Exp, scale=ISQD, bias=cbias[:, nb : nb + 1],
                        accum_out=s1p[:, nv, None],
                    )
                    if nv == 0:
                        _pop()
                s1 = smp.tile([NL, 1], F32, tag="s1")
                nc.vector.reduce_sum(s1, s1p, axis=mybir.AxisListType.X)
                ivs1 = smp.tile([NL, 1], F32, tag="ivs1")
                nc.vector.reciprocal(ivs1, s1)
                return (E, ivs1)

            # ---------------- batch 0 head ----------------
            FVTh0 = [tg00, _emit_tg(FVn0, 1)]
            vstate = {0: [FVn0, FVTh0, None]}
            vstate[0][2] = _emit_araw_E(0, FVTh0)

            # language phase part 2 (needed only from m_small onward): runs on
            # PE while the Activation engine computes batch 0's exp.
            FWVT = lp.tile([P, 6, NLB], BF)
            for mv in range(6):
                ps = tp.tile([P, NLB], F32, tag="tp")
                for ko in range(4):
                    nc.tensor.matmul(
                        ps, Wvc[:, ko, mv * P : (mv + 1) * P], FLT[:, ko, :],
                        start=(ko == 0), stop=(ko == 3),
                    )
                if mv % 2 == 0:
                    nc.vector.tensor_scalar_add(FWVT[:, mv, :], ps, c2vt[:, mv, None])
                else:
                    nc.scalar.activation(
                        FWVT[:, mv, :], ps, AF.Identity, bias=c2vt[:, mv, None]
                    )

            # cv as a row: cvr[0, b*77+l] = (fv_l @ b_vv + b_lv@b_vv)[l]
            pscv = tp.tile([1, NLB], F32, tag="tp")
            for ko in range(4):
                nc.tensor.matmul(
                    pscv, wcvt[:, ko : ko + 1], FLT[:, ko, :],
                    start=(ko == 0), stop=False,
                )
            nc.tensor.matmul(
                pscv, ccvt[:1, :1], onesr[:1, :NLB], start=False, stop=True
            )
            cvr = cp.tile([1, NLB], BF)
            nc.vector.tensor_copy(cvr, pscv)
            lph_tmp.__exit__(None, None, None)

            # ---------------- per-batch vision phase ----------------
            for b in range(BL):
                ls = b * NL

                FVn, FVTh, pre = vstate.pop(b)
                E, ivs1 = pre

                if b + 1 < BL:
                    FVn1 = _issue_load(b + 1)
                    vstate[b + 1] = [FVn1, None, None]
                _pop()

                # E^T blocks + s2 (column sums of E)
                ET = smp.tile([P, 8, NL], BF, tag="ET")
                s2 = smp.tile([P, 8], F32, tag="s2")
                for tg in range(2):
                    ps = tp.tile([P, 512], BF, tag="tp")
                    for tt in range(4):
                        t = tg * 4 + tt
                        nc.tensor.transpose(
                            ps[:, tt * P : (tt + 1) * P],
                            E[:, t * P : (t + 1) * P],
                            iden,
                        )
                    psv = ps.rearrange("p (four c) -> p four c", four=4)[:, :, :NL]
                    if tg == 0:
                        nc.scalar.activation(ET[:, 0:4, :], psv, AF.Copy)
                    else:
                        nc.vector.tensor_copy(ET[:, 4:8, :], psv)
                    nc.vector.reduce_sum(
                        s2[:, tg * 4 : (tg + 1) * 4],
                        ET[:, tg * 4 : (tg + 1) * 4, :],
                        axis=mybir.AxisListType.X,
                    )
                ivs2 = smp.tile([P, 8], F32, tag="ivs2")
                nc.vector.reciprocal(ivs2, s2)
                _pop()

                # s2 row into E row 77 (pre-multiplies b_m by s2 in the finals)
                for nv in range(2):
                    ps2 = tp.tile([1, 512], F32, tag="tp")
                    nc.tensor.matmul(
                        ps2, onesc[:NL, :], E[:NL, nv * 512 : (nv + 1) * 512],
                        start=True, stop=True,
                    )
                    if nv == 0:
                        nc.vector.tensor_copy(E[96:97, :512], ps2)
                    else:
                        nc.scalar.activation(E[96:97, 512:], ps2, AF.Copy)
                _pop()

                # h1T_un = (E @ fv)^T directly: [768, 77] in 6 chunk-rows of a
                # single 1-bank psum tile (contraction over v on partitions)
                psh = tp.tile([P, 6, NL], F32, tag="tp")
                for ko in range(6):
                    sl = psh[:, ko, :]
                    for t in range(8):
                        nc.tensor.matmul(
                            sl, FVn[:, t, ko * P : (ko + 1) * P], ET[:, t, :],
                            start=(t == 0), stop=(t == 7),
                        )
                    if ko == 1:
                        _pop()
                H1T = smp.tile([P, 6, NL], BF, tag="H1T")
                nc.scalar.activation(H1T, psh, AF.Copy)

                # fv^T transposes for the next batch fill the H1T wait
                if b + 1 < BL:
                    vstate[b + 1][1] = [_emit_tg(vstate[b + 1][0], 0)]
                _pop()

                # MT = m_small^T (1/s1 applied on copy-back); row 77 = cv
                MT = smp.tile([97, NL], BF, tag="MT")
                if b < 2:
                    # rows 77..95 are read (x0) by the Y matmul's [0:97) lhsT
                    nc.gpsimd.memset(MT[64:96, :], 0.0)
                psm = tp.tile([NL, NL], F32, tag="tp")
                for ko in range(6):
                    nc.tensor.matmul(
                        psm, H1T[:, ko, :], FWVT[:, ko, ls : ls + NL],
                        start=(ko == 0), stop=(ko == 5),
                    )
                nc.scalar.activation(MT[:NL, :], psm, AF.Identity, scale=ivs1)
                nc.vector.tensor_copy(MT[96:97, :], cvr[:, ls : ls + NL])

                # second transpose group of next batch fills the MT wait
                if b + 1 < BL:
                    vstate[b + 1][1].append(_emit_tg(vstate[b + 1][0], 1))
                _pop()

                # Y = m_small @ W_m (cv x colsum(W_m) enters via MT/WmE row 77)
                Y = YS[b % 2]
                psy = accp.tile([NL, OD], F32, tag="acc")
                for c0, cw in ((0, 512), (512, 256)):
                    nc.tensor.matmul(
                        psy[:, c0 : c0 + cw], MT, WmE[:, c0 : c0 + cw],
                        start=True, stop=True,
                    )
                nc.vector.tensor_copy(Y[:NL, :], psy)

                # enqueue this batch's 8 finals as single-tile chunks. The last
                # batch drains with no other PE work to hide PSUM-ring waits,
                # so it uses 1-bank psum pieces from the deeper tp ring.
                deep = b == BL - 1
                def _emit_final_t(b=b, E=E, Y=Y, ivs2=ivs2, deep=deep, t=0):
                    OT = outp.tile([P, OD], BF, tag="OT")
                    if deep:
                        psoA = tp.tile([P, 512], F32, tag="tp")
                        psoB = tp.tile([P, 256], F32, tag="tp")
                        nc.tensor.matmul(
                            psoA, E[:97, t * P : (t + 1) * P], Y[:, 0:512],
                            start=True, stop=True,
                        )
                        nc.tensor.matmul(
                            psoB, E[:97, t * P : (t + 1) * P], Y[:, 512:768],
                            start=True, stop=True,
                        )
                        srcs = (psoA, psoB, 512)
                    else:
                        pso = accp.tile([P, OD], F32, tag="acc")
                        for c0, cw in ((0, 512), (512, 256)):
                            nc.tensor.matmul(
                                pso[:, c0 : c0 + cw], E[:97, t * P : (t + 1) * P],
                                Y[:, c0 : c0 + cw], start=True, stop=True,
                            )
                        srcs = (pso[:, 0:384], pso[:, 384:768], 384)
                    # 2-way copy split keeps per-final latency low and
                    # balances the PSUM drain across Act and DVE
                    sc = ivs2[:, t, None]
                    nc.scalar.activation(
                        OT[:, 0 : srcs[2]], srcs[0], AF.Identity, scale=sc
                    )
                    nc.vector.tensor_scalar_mul(OT[:, srcs[2] :], srcs[1], sc)
                    nc.sync.dma_start(out_d[b, t * P : (t + 1) * P, :], OT)
                import functools as _ft
                for t in range(8):
                    finals_q.append(_ft.partial(_emit_final_t, t=t))
                if b == 0:
                    _pop()  # get the first store to the idle DMA engines early

                # next batch's a_raw + exp (Act overlaps the pops below)
                if b + 1 in vstate:
                    vstate[b + 1][2] = _emit_araw_E(b + 1, vstate[b + 1][1])
                _pop(2)

            _pop(len(finals_q))

    nc.compile()
    return nc


_NC_CACHE = None
_last_in_maps = None


def kernel(**inputs) -> np.ndarray:
    bf = ml_dtypes.bfloat16
    f32 = np.float32
    fv = np.asarray(inputs["fv"], f32)
    fl = np.asarray(inputs["fl"], f32)
    W_vk = np.asarray(inputs["W_vk"], f32)
    b_vk = np.asarray(inputs["b_vk"], f32)
    W_vv = np.asarray(inputs["W_vv"], f32)
    b_vv = np.asarray(inputs["b_vv"], f32)
    W_lk = np.asarray(inputs["W_lk"], f32)
    b_lk = np.asarray(inputs["b_lk"], f32)
    W_lv = np.asarray(inputs["W_lv"], f32)
    b_lv = np.asarray(inputs["b_lv"], f32)
    W_m = np.asarray(inputs["W_m"], f32)
    b_m = np.asarray(inputs["b_m"], f32)

    wct_pk = (W_lk @ b_vk).reshape(4, P).T
    wcvt_pk = (W_lv @ b_vv).reshape(4, P).T
    ccv = float(b_lv @ b_vv)
    consts = {
        "wkc": np.ascontiguousarray(W_lk @ W_vk.T).astype(bf),
        "wvc": np.ascontiguousarray(W_lv @ W_vv.T).astype(bf),
        "cpkb": np.concatenate(
            [wct_pk, wcvt_pk, np.full((P, 2), ccv, f32)], axis=1
        ).astype(bf),
        "cpkf": np.concatenate(
            [(W_vk @ b_lk).reshape(6, P).T, (W_vv @ b_lv).reshape(6, P).T], axis=1
        ).astype(f32),
        "wme": np.concatenate(
            [W_m, np.zeros((19, OD), f32), W_m.sum(0, keepdims=True)], axis=0
        ).astype(bf),
        "bmr": b_m[None, :].astype(bf),
        "iden": np.eye(P, dtype=bf),
    }
    fvb = fv.astype(bf)
    flb = fl.astype(bf)
    in_maps = []
    for c in range(NCORES):
        m = dict(consts)
        m["fv"] = np.ascontiguousarray(fvb[c * BL : (c + 1) * BL])
        m["fl"] = np.ascontiguousarray(flb[c * BL : (c + 1) * BL])
        in_maps.append(m)

    global _last_in_maps
    _last_in_maps = in_maps
    nc = _get_nc()
    res = run_bass_kernel_spmd(nc, in_maps, core_ids=list(range(NCORES)))
    out = np.concatenate([res.results[c]["out"] for c in range(NCORES)], axis=0)
    return np.ascontiguousarray(out, dtype=np.float32)


def _get_nc():
    global _NC_CACHE
    if _NC_CACHE is None:
        _NC_CACHE = _build()
    return _NC_CACHE



# revision 19
# speedup vs baseline: 1.0013x; 1.0013x over previous
"""Trainium2 Bass kernel for DenseLanguageGuidanceModule.

Math (per batch b):
    fk_l = fl @ W_lk + b_lk            [77, 512]
    fv-side projections are folded away algebraically:
      a_raw = (fk_l @ W_vk^T) @ fv^T + c 1^T   (/= sqrt(512)),
        c = fk_l @ b_vk  (the b_lk@b_vk scalar cancels in both softmaxes)
      fa_v  = diag(1/s1) (E @ fv) @ W_vv + b_vv,  E = exp(a_raw/sqrt(512) + c/sqrt(512))
      fm    = diag(1/s2) E^T @ (fv_l @ fa_v^T)
      out   = fm @ W_m + b_m
    where s1 = row sums of E, s2 = column sums of E.

Distribution: pure data-parallel over batch B=32 across 8 NeuronCores
(4 batches per core), weights replicated. No collectives.

All data tiles are bfloat16 (matmuls run 1 cycle/row at any free size and
input DMA bytes are halved; the kernel is DMA-bound so this matters most).
PSUM accumulation and all reductions/scales stay fp32; the output is fp32.

Software pipelining: finals (out-tile matmul + store) of batch b are split
into 8 single-tile chunks and popped into every dependency-stall slot of
batch b+1's chain, keeping PE and the DMA engines continuously busy.
"""
import sys

sys.path.insert(0, "/opt/trn_rl_repo")

import ml_dtypes
import numpy as np

import concourse.bass as bass  # noqa: E402
import concourse.tile as tile  # noqa: E402
from concourse import bacc, mybir  # noqa: E402
from concourse.bass_utils import run_bass_kernel_spmd  # noqa: E402

P = 128
NCORES = 8
B = 32
BL = B // NCORES          # 4 batches per core
NV, DV = 1024, 768        # vision tokens / dim
NL, DL = 77, 512          # language tokens / dim
D = 512                   # shared feature dim
OD = 768                  # output dim
NLB = NL * BL             # 308: l-dim stacked across local batches

BF = mybir.dt.bfloat16
F32 = mybir.dt.float32
ISQD = 1.0 / float(np.sqrt(np.float32(D)))

AF = mybir.ActivationFunctionType


def _build():
    nc = bacc.Bacc("TRN2", target_bir_lowering=False)

    fv_d = nc.dram_tensor("fv", [BL, NV, DV], BF, kind="ExternalInput")
    fl_d = nc.dram_tensor("fl", [BL, NL, DL], BF, kind="ExternalInput")
    wkc_d = nc.dram_tensor("wkc", [DL, DV], BF, kind="ExternalInput")
    wvc_d = nc.dram_tensor("wvc", [DL, DV], BF, kind="ExternalInput")
    wme_d = nc.dram_tensor("wme", [97, OD], BF, kind="ExternalInput")
    bmr_d = nc.dram_tensor("bmr", [1, OD], BF, kind="ExternalInput")
    cpkb_d = nc.dram_tensor("cpkb", [P, 10], BF, kind="ExternalInput")
    cpkf_d = nc.dram_tensor("cpkf", [P, 12], F32, kind="ExternalInput")
    iden_d = nc.dram_tensor("iden", [P, P], BF, kind="ExternalInput")
    out_d = nc.dram_tensor("out", [BL, NV, OD], BF, kind="ExternalOutput")

    with tile.TileContext(nc) as tc:
        with (
            tc.tile_pool(name="consts", bufs=1) as cp,
            tc.tile_pool(name="lph", bufs=1) as lp,
            tc.tile_pool(name="fvn", bufs=2) as fvnp,
            tc.tile_pool(name="fvt", bufs=3) as fvtp,
            tc.tile_pool(name="eb", bufs=2) as ebp,
            tc.tile_pool(name="sm", bufs=2) as smp,
            tc.tile_pool(name="outp", bufs=4) as outp,
            tc.tile_pool(name="tp", bufs=4, space="PSUM") as tp,       # 1-bank slots
            tc.tile_pool(name="acc", bufs=2, space="PSUM") as accp,    # 2-bank slots
        ):
            # ------- small constants first: keep them off the DMA critical path
            iden = cp.tile([P, P], BF)
            nc.sync.dma_start(iden, iden_d[:, :])
            onesc = cp.tile([P, 1], BF)
            nc.vector.memset(onesc, 1.0)
            onesr = cp.tile([1, NLB], BF)
            nc.vector.memset(onesr, 1.0)

            lph_tmp = tc.tile_pool(name="lphtmp", bufs=1)
            lpt = lph_tmp.__enter__()
            # fl natural layout: [128, 3, 512] (row-chunks of 128/128/52)
            FLn = lpt.tile([P, 3, DL], BF)
            fl_flat = fl_d.rearrange("b l d -> (b l) d")
            nc.sync.dma_start(FLn[:, 0, :], fl_flat[:P])
            nc.sync.dma_start(FLn[:, 1, :], fl_flat[P : 2 * P])
            nc.sync.dma_start(FLn[: NLB - 2 * P, 2, :], fl_flat[2 * P :, :])

            # fv0 first half ahead of wkc: its transposes are the first PE work
            # after FLT; wkc ahead of the packed small consts (GT needs it next)
            FVn0 = fvnp.tile([P, 8, DV], BF, tag="fvn")
            fvb0 = fv_d[0].rearrange("(t p) d -> p t d", p=P)
            nc.sync.dma_start(FVn0[:, 0:4, :], fvb0[:, 0:4, :])
            Wkc = lpt.tile([P, 4, DV], BF)
            nc.sync.dma_start(Wkc, wkc_d.rearrange("(ko p) m -> p ko m", p=P))
            nc.sync.dma_start(FVn0[:, 4:8, :], fvb0[:, 4:8, :])

            cpkb = cp.tile([P, 10], BF)
            nc.sync.dma_start(cpkb, cpkb_d[:, :])
            cpkf = cp.tile([P, 12], F32)
            nc.sync.dma_start(cpkf, cpkf_d[:, :])
            wct = cpkb[:, 0:4]
            wcvt = cpkb[:, 4:8]
            ccvt = cpkb[:1, 8:9]
            c2t = cpkf[:, 0:6]
            c2vt = cpkf[:, 6:12]
            WmE = cp.tile([97, OD], BF)
            nc.sync.dma_start(WmE, wme_d[:, :])
            bmr = cp.tile([1, OD], BF)
            nc.sync.dma_start(bmr, bmr_d[:, :])
            Wvc = lpt.tile([P, 4, DV], BF)
            nc.sync.dma_start(Wvc, wvc_d.rearrange("(ko p) m -> p ko m", p=P))

            # ---------------- language phase part 1: FLT, GT, c ----------------
            # FLT = fl_all^T  [512(D1 on p), 308]
            FLT = lpt.tile([P, 4, NLB], BF)
            for ko in range(4):
                ps = tp.tile([P, 384], BF, tag="tp")
                for i in range(3):
                    nc.tensor.transpose(
                        ps[:, i * P : (i + 1) * P],
                        FLn[:, i, ko * P : (ko + 1) * P],
                        iden,
                    )
                if ko % 2 == 0:
                    nc.vector.tensor_copy(FLT[:, ko, :], ps[:, :NLB])
                else:
                    nc.scalar.activation(FLT[:, ko, :], ps[:, :NLB], AF.Copy)

            def _emit_tg(FVn, tg):
                fvth = fvtp.tile([P, 6, 512], BF, tag="fvt")
                for ko in range(6):
                    ps = tp.tile([P, 512], BF, tag="tp")
                    for tt in range(4):
                        t = tg * 4 + tt
                        nc.tensor.transpose(
                            ps[:, tt * P : (tt + 1) * P],
                            FVn[:, t, ko * P : (ko + 1) * P],
                            iden,
                        )
                    if (ko + tg) % 2 == 0:
                        nc.vector.tensor_copy(fvth[:, ko, :], ps)
                    else:
                        nc.scalar.activation(fvth[:, ko, :], ps, AF.Copy)
                return fvth

            # fv0's first transpose group before GT: fv0-q0 lands before wkc
            tg00 = _emit_tg(FVn0, 0)

            # GT = g^T = (W_lk @ W_vk^T)^T @ fl^T + c2 : [768, 308]
            GT = lp.tile([P, 6, NLB], BF)
            for mv in range(6):
                ps = tp.tile([P, NLB], F32, tag="tp")
                for ko in range(4):
                    nc.tensor.matmul(
                        ps, Wkc[:, ko, mv * P : (mv + 1) * P], FLT[:, ko, :],
                        start=(ko == 0), stop=(ko == 3),
                    )
                if mv % 2 == 0:
                    nc.vector.tensor_scalar_add(GT[:, mv, :], ps, c2t[:, mv, None])
                else:
                    nc.scalar.activation(
                        GT[:, mv, :], ps, AF.Identity, bias=c2t[:, mv, None]
                    )

            # cbias[:, b] = (fk_l @ b_vk) * ISQD as a column per batch : [77, 4]
            psc = tp.tile([NL, 4], F32, tag="tp")
            for b in range(BL):
                ls = b * NL
                for ko in range(4):
                    nc.tensor.matmul(
                        psc[:, b : b + 1], FLT[:, ko, ls : ls + NL],
                        wct[:, ko : ko + 1],
                        start=(ko == 0), stop=(ko == 3),
                    )
            cbias = cp.tile([NL, 4], F32)
            nc.scalar.mul(cbias, psc, ISQD)

            # persistent Y ping-pong pair: Y = m_small @ W_m  [77, 768] per
            # batch, extended with row 77 = b_m (written once; rows 0..76 are
            # rewritten every batch). The finals contract E_ext^T @ Y_ext.
            YS = []
            for _i in range(2):
                _y = lp.tile([97, OD], BF, tag=f"YS{_i}")
                nc.gpsimd.memset(_y[64:96, :], 0.0)
                nc.vector.tensor_copy(_y[96:97, :], bmr)
                YS.append(_y)

            # ---------------- pipelined per-batch machinery ----------------
            finals_q = []

            def _pop(n=1):
                for _ in range(n):
                    if finals_q:
                        finals_q.pop(0)()

            def _issue_load(nb):
                FVn = fvnp.tile([P, 8, DV], BF, tag="fvn")
                fvb = fv_d[nb].rearrange("(t p) d -> p t d", p=P)
                for q in range(2):
                    nc.sync.dma_start(
                        FVn[:, 4 * q : 4 * q + 4, :], fvb[:, 4 * q : 4 * q + 4, :]
                    )
                return FVn

            def _emit_araw_E(nb, FVTh):
                nls = nb * NL
                # a_raw = g @ fv^T -> 2x 1-bank psum [77, 512] halves
                E = ebp.tile([P, NV], BF, tag="E")
                if nb < 2:
                    # rows 77..95 are read (x0) by the finals' [0:97) lhsT;
                    # zero them once per ring buffer (never written again)
                    nc.gpsimd.memset(E[64:96, :], 0.0)
                s1p = smp.tile([NL, 2], F32, tag="s1p")
                for nv in range(2):
                    sl = tp.tile([NL, 512], F32, tag="tp")
                    for ko in range(6):
                        nc.tensor.matmul(
                            sl, GT[:, ko, nls : nls + NL],
                            FVTh[nv][:, ko, :],
                            start=(ko == 0), stop=(ko == 5),
                        )
                    nc.scalar.activation(
                        E[:NL, nv * 512 : (nv + 1) * 512], sl,
                        AF.Exp, scale=ISQD, bias=cbias[:, nb : nb + 1],
                        accum_out=s1p[:, nv, None],
                    )
                    if nv == 0:
                        _pop()
                s1 = smp.tile([NL, 1], F32, tag="s1")
                nc.vector.reduce_sum(s1, s1p, axis=mybir.AxisListType.X)
                ivs1 = smp.tile([NL, 1], F32, tag="ivs1")
                nc.vector.reciprocal(ivs1, s1)
                return (E, ivs1)

            # ---------------- batch 0 head ----------------
            FVTh0 = [tg00, _emit_tg(FVn0, 1)]
            vstate = {0: [FVn0, FVTh0, None]}
            vstate[0][2] = _emit_araw_E(0, FVTh0)

            # language phase part 2 (needed only from m_small onward): runs on
            # PE while the Activation engine computes batch 0's exp.
            FWVT = lp.tile([P, 6, NLB], BF)
            for mv in range(6):
                ps = tp.tile([P, NLB], F32, tag="tp")
                for ko in range(4):
                    nc.tensor.matmul(
                        ps, Wvc[:, ko, mv * P : (mv + 1) * P], FLT[:, ko, :],
                        start=(ko == 0), stop=(ko == 3),
                    )
                if mv % 2 == 0:
                    nc.vector.tensor_scalar_add(FWVT[:, mv, :], ps, c2vt[:, mv, None])
                else:
                    nc.scalar.activation(
                        FWVT[:, mv, :], ps, AF.Identity, bias=c2vt[:, mv, None]
                    )

            # cv as a row: cvr[0, b*77+l] = (fv_l @ b_vv + b_lv@b_vv)[l]
            pscv = tp.tile([1, NLB], F32, tag="tp")
            for ko in range(4):
                nc.tensor.matmul(
                    pscv, wcvt[:, ko : ko + 1], FLT[:, ko, :],
                    start=(ko == 0), stop=False,
                )
            nc.tensor.matmul(
                pscv, ccvt[:1, :1], onesr[:1, :NLB], start=False, stop=True
            )
            cvr = cp.tile([1, NLB], BF)
            nc.vector.tensor_copy(cvr, pscv)
            lph_tmp.__exit__(None, None, None)

            # ---------------- per-batch vision phase ----------------
            for b in range(BL):
                ls = b * NL

                FVn, FVTh, pre = vstate.pop(b)
                E, ivs1 = pre

                if b + 1 < BL:
                    FVn1 = _issue_load(b + 1)
                    vstate[b + 1] = [FVn1, None, None]
                _pop()

                # E^T blocks + s2 (column sums of E)
                ET = smp.tile([P, 8, NL], BF, tag="ET")
                s2 = smp.tile([P, 8], F32, tag="s2")
                for tg in range(2):
                    ps = tp.tile([P, 512], BF, tag="tp")
                    for tt in range(4):
                        t = tg * 4 + tt
                        nc.tensor.transpose(
                            ps[:, tt * P : (tt + 1) * P],
                            E[:, t * P : (t + 1) * P],
                            iden,
                        )
                    psv = ps.rearrange("p (four c) -> p four c", four=4)[:, :, :NL]
                    if tg == 0:
                        nc.scalar.activation(ET[:, 0:4, :], psv, AF.Copy)
                    else:
                        nc.vector.tensor_copy(ET[:, 4:8, :], psv)
                    nc.vector.reduce_sum(
                        s2[:, tg * 4 : (tg + 1) * 4],
                        ET[:, tg * 4 : (tg + 1) * 4, :],
                        axis=mybir.AxisListType.X,
                    )
                ivs2 = smp.tile([P, 8], F32, tag="ivs2")
                nc.vector.reciprocal(ivs2, s2)
                _pop()

                # s2 row into E row 77 (pre-multiplies b_m by s2 in the finals)
                for nv in range(2):
                    ps2 = tp.tile([1, 512], F32, tag="tp")
                    nc.tensor.matmul(
                        ps2, onesc[:NL, :], E[:NL, nv * 512 : (nv + 1) * 512],
                        start=True, stop=True,
                    )
                    if nv == 0:
                        nc.vector.tensor_copy(E[96:97, :512], ps2)
                    else:
                        nc.scalar.activation(E[96:97, 512:], ps2, AF.Copy)
                _pop()

                # h1T_un = (E @ fv)^T directly: [768, 77] in 6 chunk-rows of a
                # single 1-bank psum tile (contraction over v on partitions)
                psh = tp.tile([P, 6, NL], F32, tag="tp")
                for ko in range(6):
                    sl = psh[:, ko, :]
                    for t in range(8):
                        nc.tensor.matmul(
                            sl, FVn[:, t, ko * P : (ko + 1) * P], ET[:, t, :],
                            start=(t == 0), stop=(t == 7),
                        )
                    if ko == 1:
                        _pop()
                H1T = smp.tile([P, 6, NL], BF, tag="H1T")
                nc.scalar.activation(H1T, psh, AF.Copy)

                # fv^T transposes for the next batch fill the H1T wait
                if b + 1 < BL:
                    vstate[b + 1][1] = [_emit_tg(vstate[b + 1][0], 0)]
                _pop()

                # MT = m_small^T (1/s1 applied on copy-back); row 77 = cv
                MT = smp.tile([97, NL], BF, tag="MT")
                if b < 2:
                    # rows 77..95 are read (x0) by the Y matmul's [0:97) lhsT
                    nc.gpsimd.memset(MT[64:96, :], 0.0)
                psm = tp.tile([NL, NL], F32, tag="tp")
                for ko in range(6):
                    nc.tensor.matmul(
                        psm, H1T[:, ko, :], FWVT[:, ko, ls : ls + NL],
                        start=(ko == 0), stop=(ko == 5),
                    )
                nc.scalar.activation(MT[:NL, :], psm, AF.Identity, scale=ivs1)
                nc.vector.tensor_copy(MT[96:97, :], cvr[:, ls : ls + NL])

                # second transpose group of next batch fills the MT wait
                if b + 1 < BL:
                    vstate[b + 1][1].append(_emit_tg(vstate[b + 1][0], 1))
                _pop()

                # Y = m_small @ W_m (cv x colsum(W_m) enters via MT/WmE row 77)
                Y = YS[b % 2]
                psy = accp.tile([NL, OD], F32, tag="acc")
                for c0, cw in ((0, 512), (512, 256)):
                    nc.tensor.matmul(
                        psy[:, c0 : c0 + cw], MT, WmE[:, c0 : c0 + cw],
                        start=True, stop=True,
                    )
                nc.vector.tensor_copy(Y[:NL, :], psy)

                # enqueue this batch's 8 finals as single-tile chunks. The last
                # batch drains with no other PE work to hide PSUM-ring waits,
                # so it uses 1-bank psum pieces from the deeper tp ring.
                deep = b == BL - 1
                def _emit_final_t(b=b, E=E, Y=Y, ivs2=ivs2, deep=deep, t=0):
                    OT = outp.tile([P, OD], BF, tag="OT")
                    sc = ivs2[:, t, None]
                    if deep:
                        # tail is latency-bound: split each copy across both
                        # engines; alternate tp-pairs and acc tiles for a
                        # deeper psum ring
                        if t % 2 == 0:
                            pso = accp.tile([P, OD], F32, tag="acc")
                            pA, pB = pso[:, 0:512], pso[:, 512:768]
                        else:
                            pA = tp.tile([P, 512], F32, tag="tp")
                            pB = tp.tile([P, 256], F32, tag="tp")
                        nc.tensor.matmul(
                            pA, E[:97, t * P : (t + 1) * P], Y[:, 0:512],
                            start=True, stop=True,
                        )
                        nc.tensor.matmul(
                            pB, E[:97, t * P : (t + 1) * P], Y[:, 512:768],
                            start=True, stop=True,
                        )
                        nc.scalar.activation(
                            OT[:, 0:512], pA, AF.Identity, scale=sc
                        )
                        nc.vector.tensor_scalar_mul(OT[:, 512:], pB, sc)
                    else:
                        # mid-kernel is throughput-bound: one whole copy,
                        # alternating engines
                        pso = accp.tile([P, OD], F32, tag="acc")
                        for c0, cw in ((0, 512), (512, 256)):
                            nc.tensor.matmul(
                                pso[:, c0 : c0 + cw], E[:97, t * P : (t + 1) * P],
                                Y[:, c0 : c0 + cw], start=True, stop=True,
                            )
                        if t % 2 == 0:
                            nc.vector.tensor_scalar_mul(OT, pso, sc)
                        else:
                            nc.scalar.activation(
                                OT, pso, AF.Identity, scale=sc
                            )
                    nc.sync.dma_start(out_d[b, t * P : (t + 1) * P, :], OT)
                import functools as _ft
                for t in range(8):
                    finals_q.append(_ft.partial(_emit_final_t, t=t))
                if b == 0:
                    _pop()  # get the first store to the idle DMA engines early

                # next batch's a_raw + exp (Act overlaps the pops below)
                if b + 1 in vstate:
                    vstate[b + 1][2] = _emit_araw_E(b + 1, vstate[b + 1][1])
                _pop(2)

            _pop(len(finals_q))

    nc.compile()
    return nc


_NC_CACHE = None
_last_in_maps = None


def kernel(**inputs) -> np.ndarray:
    bf = ml_dtypes.bfloat16
    f32 = np.float32
    fv = np.asarray(inputs["fv"], f32)
    fl = np.asarray(inputs["fl"], f32)
    W_vk = np.asarray(inputs["W_vk"], f32)
    b_vk = np.asarray(inputs["b_vk"], f32)
    W_vv = np.asarray(inputs["W_vv"], f32)
    b_vv = np.asarray(inputs["b_vv"], f32)
    W_lk = np.asarray(inputs["W_lk"], f32)
    b_lk = np.asarray(inputs["b_lk"], f32)
    W_lv = np.asarray(inputs["W_lv"], f32)
    b_lv = np.asarray(inputs["b_lv"], f32)
    W_m = np.asarray(inputs["W_m"], f32)
    b_m = np.asarray(inputs["b_m"], f32)

    wct_pk = (W_lk @ b_vk).reshape(4, P).T
    wcvt_pk = (W_lv @ b_vv).reshape(4, P).T
    ccv = float(b_lv @ b_vv)
    consts = {
        "wkc": np.ascontiguousarray(W_lk @ W_vk.T).astype(bf),
        "wvc": np.ascontiguousarray(W_lv @ W_vv.T).astype(bf),
        "cpkb": np.concatenate(
            [wct_pk, wcvt_pk, np.full((P, 2), ccv, f32)], axis=1
        ).astype(bf),
        "cpkf": np.concatenate(
            [(W_vk @ b_lk).reshape(6, P).T, (W_vv @ b_lv).reshape(6, P).T], axis=1
        ).astype(f32),
        "wme": np.concatenate(
            [W_m, np.zeros((19, OD), f32), W_m.sum(0, keepdims=True)], axis=0
        ).astype(bf),
        "bmr": b_m[None, :].astype(bf),
        "iden": np.eye(P, dtype=bf),
    }
    fvb = fv.astype(bf)
    flb = fl.astype(bf)
    in_maps = []
    for c in range(NCORES):
        m = dict(consts)
        m["fv"] = np.ascontiguousarray(fvb[c * BL : (c + 1) * BL])
        m["fl"] = np.ascontiguousarray(flb[c * BL : (c + 1) * BL])
        in_maps.append(m)

    global _last_in_maps
    _last_in_maps = in_maps
    nc = _get_nc()
    res = run_bass_kernel_spmd(nc, in_maps, core_ids=list(range(NCORES)))
    out = np.concatenate([res.results[c]["out"] for c in range(NCORES)], axis=0)
    return np.ascontiguousarray(out, dtype=np.float32)


def _get_nc():
    global _NC_CACHE
    if _NC_CACHE is None:
        _NC_CACHE = _build()
    return _NC_CACHE



# revision 20
# speedup vs baseline: 1.0103x; 1.0090x over previous
"""Trainium2 Bass kernel for DenseLanguageGuidanceModule.

Math (per batch b):
    fk_l = fl @ W_lk + b_lk            [77, 512]
    fv-side projections are folded away algebraically:
      a_raw = (fk_l @ W_vk^T) @ fv^T + c 1^T   (/= sqrt(512)),
        c = fk_l @ b_vk  (the b_lk@b_vk scalar cancels in both softmaxes)
      fa_v  = diag(1/s1) (E @ fv) @ W_vv + b_vv,  E = exp(a_raw/sqrt(512) + c/sqrt(512))
      fm    = diag(1/s2) E^T @ (fv_l @ fa_v^T)
      out   = fm @ W_m + b_m
    where s1 = row sums of E, s2 = column sums of E.

Distribution: pure data-parallel over batch B=32 across 8 NeuronCores
(4 batches per core), weights replicated. No collectives.

All data tiles are bfloat16 (matmuls run 1 cycle/row at any free size and
input DMA bytes are halved; the kernel is DMA-bound so this matters most).
PSUM accumulation and all reductions/scales stay fp32; the output is fp32.

Software pipelining: finals (out-tile matmul + store) of batch b are split
into 8 single-tile chunks and popped into every dependency-stall slot of
batch b+1's chain, keeping PE and the DMA engines continuously busy.
"""
import sys

sys.path.insert(0, "/opt/trn_rl_repo")

import ml_dtypes
import numpy as np

import concourse.bass as bass  # noqa: E402
import concourse.tile as tile  # noqa: E402
from concourse import bacc, mybir  # noqa: E402
from concourse.bass_utils import run_bass_kernel_spmd  # noqa: E402

P = 128
NCORES = 8
B = 32
BL = B // NCORES          # 4 batches per core
NV, DV = 1024, 768        # vision tokens / dim
NL, DL = 77, 512          # language tokens / dim
D = 512                   # shared feature dim
OD = 768                  # output dim
NLB = NL * BL             # 308: l-dim stacked across local batches

BF = mybir.dt.bfloat16
F32 = mybir.dt.float32
ISQD = 1.0 / float(np.sqrt(np.float32(D)))

AF = mybir.ActivationFunctionType


def _build():
    nc = bacc.Bacc("TRN2", target_bir_lowering=False)

    fv_d = nc.dram_tensor("fv", [BL, NV, DV], BF, kind="ExternalInput")
    fl_d = nc.dram_tensor("fl", [BL, NL, DL], BF, kind="ExternalInput")
    wkc_d = nc.dram_tensor("wkc", [DL, DV], BF, kind="ExternalInput")
    wvc_d = nc.dram_tensor("wvc", [DL, DV], BF, kind="ExternalInput")
    wme_d = nc.dram_tensor("wme", [97, OD], BF, kind="ExternalInput")
    bmr_d = nc.dram_tensor("bmr", [1, OD], BF, kind="ExternalInput")
    cpkb_d = nc.dram_tensor("cpkb", [P, 10], BF, kind="ExternalInput")
    cpkf_d = nc.dram_tensor("cpkf", [P, 12], F32, kind="ExternalInput")
    iden_d = nc.dram_tensor("iden", [P, P], BF, kind="ExternalInput")
    out_d = nc.dram_tensor("out", [BL, NV, OD], BF, kind="ExternalOutput")

    with tile.TileContext(nc) as tc:
        with (
            tc.tile_pool(name="consts", bufs=1) as cp,
            tc.tile_pool(name="lph", bufs=1) as lp,
            tc.tile_pool(name="fvn", bufs=2) as fvnp,
            tc.tile_pool(name="fvt", bufs=3) as fvtp,
            tc.tile_pool(name="eb", bufs=2) as ebp,
            tc.tile_pool(name="sm", bufs=2) as smp,
            tc.tile_pool(name="outp", bufs=4) as outp,
            tc.tile_pool(name="tp", bufs=4, space="PSUM") as tp,       # 1-bank slots
            tc.tile_pool(name="acc", bufs=2, space="PSUM") as accp,    # 2-bank slots
        ):
            # ------- small constants first: keep them off the DMA critical path
            iden = cp.tile([P, P], BF)
            nc.sync.dma_start(iden, iden_d[:, :])
            onesc = cp.tile([P, 1], BF)
            nc.vector.memset(onesc, 1.0)
            onesr = cp.tile([1, NLB], BF)
            nc.vector.memset(onesr, 1.0)

            lph_tmp = tc.tile_pool(name="lphtmp", bufs=1)
            lpt = lph_tmp.__enter__()
            # fl natural layout: [128, 3, 512] (row-chunks of 128/128/52)
            FLn = lpt.tile([P, 3, DL], BF)
            fl_flat = fl_d.rearrange("b l d -> (b l) d")
            nc.sync.dma_start(
                FLn[:, 0:2, :], fl_flat[: 2 * P].rearrange("(t p) d -> p t d", p=P)
            )
            nc.sync.dma_start(FLn[: NLB - 2 * P, 2, :], fl_flat[2 * P :, :])

            # fv0 first half ahead of wkc: its transposes are the first PE work
            # after FLT; wkc ahead of the packed small consts (GT needs it next)
            FVn0 = fvnp.tile([P, 8, DV], BF, tag="fvn")
            fvb0 = fv_d[0].rearrange("(t p) d -> p t d", p=P)
            nc.sync.dma_start(FVn0[:, 0:4, :], fvb0[:, 0:4, :])
            Wkc = lpt.tile([P, 4, DV], BF)
            nc.sync.dma_start(Wkc, wkc_d.rearrange("(ko p) m -> p ko m", p=P))
            nc.sync.dma_start(FVn0[:, 4:8, :], fvb0[:, 4:8, :])

            cpkb = cp.tile([P, 10], BF)
            nc.sync.dma_start(cpkb, cpkb_d[:, :])
            cpkf = cp.tile([P, 12], F32)
            nc.sync.dma_start(cpkf, cpkf_d[:, :])
            wct = cpkb[:, 0:4]
            wcvt = cpkb[:, 4:8]
            ccvt = cpkb[:1, 8:9]
            c2t = cpkf[:, 0:6]
            c2vt = cpkf[:, 6:12]
            WmE = cp.tile([97, OD], BF)
            nc.sync.dma_start(WmE, wme_d[:, :])
            bmr = cp.tile([1, OD], BF)
            nc.sync.dma_start(bmr, bmr_d[:, :])
            Wvc = lpt.tile([P, 4, DV], BF)
            nc.sync.dma_start(Wvc, wvc_d.rearrange("(ko p) m -> p ko m", p=P))

            # ---------------- language phase part 1: FLT, GT, c ----------------
            # FLT = fl_all^T  [512(D1 on p), 308]
            FLT = lpt.tile([P, 4, NLB], BF)
            for ko in range(4):
                ps = tp.tile([P, 384], BF, tag="tp")
                for i in range(3):
                    nc.tensor.transpose(
                        ps[:, i * P : (i + 1) * P],
                        FLn[:, i, ko * P : (ko + 1) * P],
                        iden,
                    )
                if ko % 2 == 0:
                    nc.vector.tensor_copy(FLT[:, ko, :], ps[:, :NLB])
                else:
                    nc.scalar.activation(FLT[:, ko, :], ps[:, :NLB], AF.Copy)

            def _emit_tg(FVn, tg):
                fvth = fvtp.tile([P, 6, 512], BF, tag="fvt")
                for ko in range(6):
                    ps = tp.tile([P, 512], BF, tag="tp")
                    for tt in range(4):
                        t = tg * 4 + tt
                        nc.tensor.transpose(
                            ps[:, tt * P : (tt + 1) * P],
                            FVn[:, t, ko * P : (ko + 1) * P],
                            iden,
                        )
                    if (ko + tg) % 2 == 0:
                        nc.vector.tensor_copy(fvth[:, ko, :], ps)
                    else:
                        nc.scalar.activation(fvth[:, ko, :], ps, AF.Copy)
                return fvth

            # fv0's first transpose group before GT: fv0-q0 lands before wkc
            tg00 = _emit_tg(FVn0, 0)

            # GT = g^T = (W_lk @ W_vk^T)^T @ fl^T + c2 : [768, 308]
            GT = lp.tile([P, 6, NLB], BF)
            for mv in range(6):
                ps = tp.tile([P, NLB], F32, tag="tp")
                for ko in range(4):
                    nc.tensor.matmul(
                        ps, Wkc[:, ko, mv * P : (mv + 1) * P], FLT[:, ko, :],
                        start=(ko == 0), stop=(ko == 3),
                    )
                if mv % 2 == 0:
                    nc.vector.tensor_scalar_add(GT[:, mv, :], ps, c2t[:, mv, None])
                else:
                    nc.scalar.activation(
                        GT[:, mv, :], ps, AF.Identity, bias=c2t[:, mv, None]
                    )

            # cbias[:, b] = (fk_l @ b_vk) * ISQD as a column per batch : [77, 4]
            psc = tp.tile([NL, 4], F32, tag="tp")
            for b in range(BL):
                ls = b * NL
                for ko in range(4):
                    nc.tensor.matmul(
                        psc[:, b : b + 1], FLT[:, ko, ls : ls + NL],
                        wct[:, ko : ko + 1],
                        start=(ko == 0), stop=(ko == 3),
                    )
            cbias = cp.tile([NL, 4], F32)
            nc.scalar.mul(cbias, psc, ISQD)

            # persistent Y ping-pong pair: Y = m_small @ W_m  [77, 768] per
            # batch, extended with row 77 = b_m (written once; rows 0..76 are
            # rewritten every batch). The finals contract E_ext^T @ Y_ext.
            YS = []
            for _i in range(2):
                _y = lp.tile([97, OD], BF, tag=f"YS{_i}")
                nc.gpsimd.memset(_y[64:96, :], 0.0)
                nc.vector.tensor_copy(_y[96:97, :], bmr)
                YS.append(_y)

            # ---------------- pipelined per-batch machinery ----------------
            finals_q = []

            def _pop(n=1):
                for _ in range(n):
                    if finals_q:
                        finals_q.pop(0)()

            def _issue_load(nb):
                FVn = fvnp.tile([P, 8, DV], BF, tag="fvn")
                fvb = fv_d[nb].rearrange("(t p) d -> p t d", p=P)
                for q in range(2):
                    nc.sync.dma_start(
                        FVn[:, 4 * q : 4 * q + 4, :], fvb[:, 4 * q : 4 * q + 4, :]
                    )
                return FVn

            def _emit_araw_E(nb, FVTh):
                nls = nb * NL
                # a_raw = g @ fv^T -> 2x 1-bank psum [77, 512] halves
                E = ebp.tile([P, NV], BF, tag="E")
                if nb < 2:
                    # rows 77..95 are read (x0) by the finals' [0:97) lhsT;
                    # zero them once per ring buffer (never written again)
                    nc.gpsimd.memset(E[64:96, :], 0.0)
                s1p = smp.tile([NL, 2], F32, tag="s1p")
                for nv in range(2):
                    sl = tp.tile([NL, 512], F32, tag="tp")
                    for ko in range(6):
                        nc.tensor.matmul(
                            sl, GT[:, ko, nls : nls + NL],
                            FVTh[nv][:, ko, :],
                            start=(ko == 0), stop=(ko == 5),
                        )
                    nc.scalar.activation(
                        E[:NL, nv * 512 : (nv + 1) * 512], sl,
                        AF.Exp, scale=ISQD, bias=cbias[:, nb : nb + 1],
                        accum_out=s1p[:, nv, None],
                    )
                    if nv == 0:
                        _pop()
                s1 = smp.tile([NL, 1], F32, tag="s1")
                nc.vector.reduce_sum(s1, s1p, axis=mybir.AxisListType.X)
                ivs1 = smp.tile([NL, 1], F32, tag="ivs1")
                nc.vector.reciprocal(ivs1, s1)
                return (E, ivs1)

            # ---------------- batch 0 head ----------------
            FVTh0 = [tg00, _emit_tg(FVn0, 1)]
            vstate = {0: [FVn0, FVTh0, None]}
            vstate[0][2] = _emit_araw_E(0, FVTh0)

            # language phase part 2 (needed only from m_small onward): runs on
            # PE while the Activation engine computes batch 0's exp.
            FWVT = lp.tile([P, 6, NLB], BF)
            for mv in range(6):
                ps = tp.tile([P, NLB], F32, tag="tp")
                for ko in range(4):
                    nc.tensor.matmul(
                        ps, Wvc[:, ko, mv * P : (mv + 1) * P], FLT[:, ko, :],
                        start=(ko == 0), stop=(ko == 3),
                    )
                if mv % 2 == 0:
                    nc.vector.tensor_scalar_add(FWVT[:, mv, :], ps, c2vt[:, mv, None])
                else:
                    nc.scalar.activation(
                        FWVT[:, mv, :], ps, AF.Identity, bias=c2vt[:, mv, None]
                    )

            # cv as a row: cvr[0, b*77+l] = (fv_l @ b_vv + b_lv@b_vv)[l]
            pscv = tp.tile([1, NLB], F32, tag="tp")
            for ko in range(4):
                nc.tensor.matmul(
                    pscv, wcvt[:, ko : ko + 1], FLT[:, ko, :],
                    start=(ko == 0), stop=False,
                )
            nc.tensor.matmul(
                pscv, ccvt[:1, :1], onesr[:1, :NLB], start=False, stop=True
            )
            cvr = cp.tile([1, NLB], BF)
            nc.vector.tensor_copy(cvr, pscv)
            lph_tmp.__exit__(None, None, None)

            # ---------------- per-batch vision phase ----------------
            for b in range(BL):
                ls = b * NL

                FVn, FVTh, pre = vstate.pop(b)
                E, ivs1 = pre

                if b + 1 < BL:
                    FVn1 = _issue_load(b + 1)
                    vstate[b + 1] = [FVn1, None, None]
                _pop()

                # E^T blocks + s2 (column sums of E)
                ET = smp.tile([P, 8, NL], BF, tag="ET")
                s2 = smp.tile([P, 8], F32, tag="s2")
                for tg in range(2):
                    ps = tp.tile([P, 512], BF, tag="tp")
                    for tt in range(4):
                        t = tg * 4 + tt
                        nc.tensor.transpose(
                            ps[:, tt * P : (tt + 1) * P],
                            E[:, t * P : (t + 1) * P],
                            iden,
                        )
                    psv = ps.rearrange("p (four c) -> p four c", four=4)[:, :, :NL]
                    if tg == 0:
                        nc.scalar.activation(ET[:, 0:4, :], psv, AF.Copy)
                    else:
                        nc.vector.tensor_copy(ET[:, 4:8, :], psv)
                    nc.vector.reduce_sum(
                        s2[:, tg * 4 : (tg + 1) * 4],
                        ET[:, tg * 4 : (tg + 1) * 4, :],
                        axis=mybir.AxisListType.X,
                    )
                ivs2 = smp.tile([P, 8], F32, tag="ivs2")
                nc.vector.reciprocal(ivs2, s2)
                _pop()

                # s2 row into E row 77 (pre-multiplies b_m by s2 in the finals)
                for nv in range(2):
                    ps2 = tp.tile([1, 512], F32, tag="tp")
                    nc.tensor.matmul(
                        ps2, onesc[:NL, :], E[:NL, nv * 512 : (nv + 1) * 512],
                        start=True, stop=True,
                    )
                    if nv == 0:
                        nc.vector.tensor_copy(E[96:97, :512], ps2)
                    else:
                        nc.scalar.activation(E[96:97, 512:], ps2, AF.Copy)
                _pop()

                # h1T_un = (E @ fv)^T directly: [768, 77] in 6 chunk-rows of a
                # single 1-bank psum tile (contraction over v on partitions)
                psh = tp.tile([P, 6, NL], F32, tag="tp")
                for ko in range(6):
                    sl = psh[:, ko, :]
                    for t in range(8):
                        nc.tensor.matmul(
                            sl, FVn[:, t, ko * P : (ko + 1) * P], ET[:, t, :],
                            start=(t == 0), stop=(t == 7),
                        )
                    if ko == 1:
                        _pop()
                H1T = smp.tile([P, 6, NL], BF, tag="H1T")
                nc.scalar.activation(H1T, psh, AF.Copy)

                # fv^T transposes for the next batch fill the H1T wait
                if b + 1 < BL:
                    vstate[b + 1][1] = [_emit_tg(vstate[b + 1][0], 0)]
                _pop()

                # MT = m_small^T (1/s1 applied on copy-back); row 77 = cv
                MT = smp.tile([97, NL], BF, tag="MT")
                if b < 2:
                    # rows 77..95 are read (x0) by the Y matmul's [0:97) lhsT
                    nc.gpsimd.memset(MT[64:96, :], 0.0)
                psm = tp.tile([NL, NL], F32, tag="tp")
                for ko in range(6):
                    nc.tensor.matmul(
                        psm, H1T[:, ko, :], FWVT[:, ko, ls : ls + NL],
                        start=(ko == 0), stop=(ko == 5),
                    )
                nc.scalar.activation(MT[:NL, :], psm, AF.Identity, scale=ivs1)
                nc.vector.tensor_copy(MT[96:97, :], cvr[:, ls : ls + NL])

                # second transpose group of next batch fills the MT wait
                if b + 1 < BL:
                    vstate[b + 1][1].append(_emit_tg(vstate[b + 1][0], 1))
                _pop()

                # Y = m_small @ W_m (cv x colsum(W_m) enters via MT/WmE row 77)
                Y = YS[b % 2]
                psy = accp.tile([NL, OD], F32, tag="acc")
                for c0, cw in ((0, 512), (512, 256)):
                    nc.tensor.matmul(
                        psy[:, c0 : c0 + cw], MT, WmE[:, c0 : c0 + cw],
                        start=True, stop=True,
                    )
                nc.vector.tensor_copy(Y[:NL, :], psy)

                # enqueue this batch's 8 finals as single-tile chunks. The last
                # batch drains with no other PE work to hide PSUM-ring waits,
                # so it uses 1-bank psum pieces from the deeper tp ring.
                deep = b == BL - 1
                def _emit_final_t(b=b, E=E, Y=Y, ivs2=ivs2, deep=deep, t=0):
                    OT = outp.tile([P, OD], BF, tag="OT")
                    sc = ivs2[:, t, None]
                    if deep:
                        # tail is latency-bound: split each copy across both
                        # engines; alternate tp-pairs and acc tiles for a
                        # deeper psum ring
                        if t % 2 == 0:
                            pso = accp.tile([P, OD], F32, tag="acc")
                            pA, pB = pso[:, 0:512], pso[:, 512:768]
                        else:
                            pA = tp.tile([P, 512], F32, tag="tp")
                            pB = tp.tile([P, 256], F32, tag="tp")
                        nc.tensor.matmul(
                            pA, E[:97, t * P : (t + 1) * P], Y[:, 0:512],
                            start=True, stop=True,
                        )
                        nc.tensor.matmul(
                            pB, E[:97, t * P : (t + 1) * P], Y[:, 512:768],
                            start=True, stop=True,
                        )
                        nc.scalar.activation(
                            OT[:, 0:512], pA, AF.Identity, scale=sc
                        )
                        nc.vector.tensor_scalar_mul(OT[:, 512:], pB, sc)
                    else:
                        # mid-kernel is throughput-bound: one whole copy,
                        # alternating engines
                        pso = accp.tile([P, OD], F32, tag="acc")
                        for c0, cw in ((0, 512), (512, 256)):
                            nc.tensor.matmul(
                                pso[:, c0 : c0 + cw], E[:97, t * P : (t + 1) * P],
                                Y[:, c0 : c0 + cw], start=True, stop=True,
                            )
                        if t % 2 == 0:
                            nc.vector.tensor_scalar_mul(OT, pso, sc)
                        else:
                            nc.scalar.activation(
                                OT, pso, AF.Identity, scale=sc
                            )
                    nc.sync.dma_start(out_d[b, t * P : (t + 1) * P, :], OT)
                import functools as _ft
                for t in range(8):
                    finals_q.append(_ft.partial(_emit_final_t, t=t))
                if b == 0:
                    _pop()  # get the first store to the idle DMA engines early

                # next batch's a_raw + exp (Act overlaps the pops below)
                if b + 1 in vstate:
                    vstate[b + 1][2] = _emit_araw_E(b + 1, vstate[b + 1][1])
                _pop(2)

            _pop(len(finals_q))

    nc.compile()
    return nc


_NC_CACHE = None
_last_in_maps = None


def kernel(**inputs) -> np.ndarray:
    bf = ml_dtypes.bfloat16
    f32 = np.float32
    fv = np.asarray(inputs["fv"], f32)
    fl = np.asarray(inputs["fl"], f32)
    W_vk = np.asarray(inputs["W_vk"], f32)
    b_vk = np.asarray(inputs["b_vk"], f32)
    W_vv = np.asarray(inputs["W_vv"], f32)
    b_vv = np.asarray(inputs["b_vv"], f32)
    W_lk = np.asarray(inputs["W_lk"], f32)
    b_lk = np.asarray(inputs["b_lk"], f32)
    W_lv = np.asarray(inputs["W_lv"], f32)
    b_lv = np.asarray(inputs["b_lv"], f32)
    W_m = np.asarray(inputs["W_m"], f32)
    b_m = np.asarray(inputs["b_m"], f32)

    wct_pk = (W_lk @ b_vk).reshape(4, P).T
    wcvt_pk = (W_lv @ b_vv).reshape(4, P).T
    ccv = float(b_lv @ b_vv)
    consts = {
        "wkc": np.ascontiguousarray(W_lk @ W_vk.T).astype(bf),
        "wvc": np.ascontiguousarray(W_lv @ W_vv.T).astype(bf),
        "cpkb": np.concatenate(
            [wct_pk, wcvt_pk, np.full((P, 2), ccv, f32)], axis=1
        ).astype(bf),
        "cpkf": np.concatenate(
            [(W_vk @ b_lk).reshape(6, P).T, (W_vv @ b_lv).reshape(6, P).T], axis=1
        ).astype(f32),
        "wme": np.concatenate(
            [W_m, np.zeros((19, OD), f32), W_m.sum(0, keepdims=True)], axis=0
        ).astype(bf),
        "bmr": b_m[None, :].astype(bf),
        "iden": np.eye(P, dtype=bf),
    }
    fvb = fv.astype(bf)
    flb = fl.astype(bf)
    in_maps = []
    for c in range(NCORES):
        m = dict(consts)
        m["fv"] = np.ascontiguousarray(fvb[c * BL : (c + 1) * BL])
        m["fl"] = np.ascontiguousarray(flb[c * BL : (c + 1) * BL])
        in_maps.append(m)

    global _last_in_maps
    _last_in_maps = in_maps
    nc = _get_nc()
    res = run_bass_kernel_spmd(nc, in_maps, core_ids=list(range(NCORES)))
    out = np.concatenate([res.results[c]["out"] for c in range(NCORES)], axis=0)
    return np.ascontiguousarray(out, dtype=np.float32)


def _get_nc():
    global _NC_CACHE
    if _NC_CACHE is None:
        _NC_CACHE = _build()
    return _NC_CACHE



# revision 21
# speedup vs baseline: 1.0287x; 1.0182x over previous
"""Trainium2 Bass kernel for DenseLanguageGuidanceModule.

Math (per batch b):
    fk_l = fl @ W_lk + b_lk            [77, 512]
    fv-side projections are folded away algebraically:
      a_raw = (fk_l @ W_vk^T) @ fv^T + c 1^T   (/= sqrt(512)),
        c = fk_l @ b_vk  (the b_lk@b_vk scalar cancels in both softmaxes)
      fa_v  = diag(1/s1) (E @ fv) @ W_vv + b_vv,  E = exp(a_raw/sqrt(512) + c/sqrt(512))
      fm    = diag(1/s2) E^T @ (fv_l @ fa_v^T)
      out   = fm @ W_m + b_m
    where s1 = row sums of E, s2 = column sums of E.

Distribution: pure data-parallel over batch B=32 across 8 NeuronCores
(4 batches per core), weights replicated. No collectives.

All data tiles are bfloat16 (matmuls run 1 cycle/row at any free size and
input DMA bytes are halved; the kernel is DMA-bound so this matters most).
PSUM accumulation and all reductions/scales stay fp32; the output is fp32.

Software pipelining: finals (out-tile matmul + store) of batch b are split
into 8 single-tile chunks and popped into every dependency-stall slot of
batch b+1's chain, keeping PE and the DMA engines continuously busy.
"""
import sys

sys.path.insert(0, "/opt/trn_rl_repo")

import ml_dtypes
import numpy as np

import concourse.bass as bass  # noqa: E402
import concourse.tile as tile  # noqa: E402
from concourse import bacc, mybir  # noqa: E402
from concourse.bass_utils import run_bass_kernel_spmd  # noqa: E402

P = 128
NCORES = 8
B = 32
BL = B // NCORES          # 4 batches per core
NV, DV = 1024, 768        # vision tokens / dim
NL, DL = 77, 512          # language tokens / dim
D = 512                   # shared feature dim
OD = 768                  # output dim
NLB = NL * BL             # 308: l-dim stacked across local batches

BF = mybir.dt.bfloat16
F32 = mybir.dt.float32
ISQD = 1.0 / float(np.sqrt(np.float32(D)))

AF = mybir.ActivationFunctionType


def _build():
    nc = bacc.Bacc("TRN2", target_bir_lowering=False)

    fv_d = nc.dram_tensor("fv", [BL, NV, DV], BF, kind="ExternalInput")
    fl_d = nc.dram_tensor("fl", [BL, NL, DL], BF, kind="ExternalInput")
    wkc_d = nc.dram_tensor("wkc", [DL, DV], BF, kind="ExternalInput")
    wvc_d = nc.dram_tensor("wvc", [DL, DV], BF, kind="ExternalInput")
    wme_d = nc.dram_tensor("wme", [97, OD], BF, kind="ExternalInput")
    bmr_d = nc.dram_tensor("bmr", [1, OD], BF, kind="ExternalInput")
    cpkb_d = nc.dram_tensor("cpkb", [P, 10], BF, kind="ExternalInput")
    cpkf_d = nc.dram_tensor("cpkf", [P, 12], F32, kind="ExternalInput")
    iden_d = nc.dram_tensor("iden", [P, P], BF, kind="ExternalInput")
    out_d = nc.dram_tensor("out", [BL, NV, OD], BF, kind="ExternalOutput")

    with tile.TileContext(nc) as tc:
        with (
            tc.tile_pool(name="consts", bufs=1) as cp,
            tc.tile_pool(name="lph", bufs=1) as lp,
            tc.tile_pool(name="fvn", bufs=2) as fvnp,
            tc.tile_pool(name="fvt", bufs=3) as fvtp,
            tc.tile_pool(name="eb", bufs=2) as ebp,
            tc.tile_pool(name="sm", bufs=2) as smp,
            tc.tile_pool(name="outp", bufs=4) as outp,
            tc.tile_pool(name="tp", bufs=4, space="PSUM") as tp,       # 1-bank slots
            tc.tile_pool(name="acc", bufs=2, space="PSUM") as accp,    # 2-bank slots
        ):
            # ------- small constants first: keep them off the DMA critical path
            iden = cp.tile([P, P], BF)
            nc.sync.dma_start(iden, iden_d[:, :])
            onesc = cp.tile([P, 1], BF)
            nc.vector.memset(onesc, 1.0)
            onesr = cp.tile([1, NLB], BF)
            nc.vector.memset(onesr, 1.0)

            lph_tmp = tc.tile_pool(name="lphtmp", bufs=1)
            lpt = lph_tmp.__enter__()
            # fl natural layout: [128, 3, 512] (row-chunks of 128/128/52)
            FLn = lpt.tile([P, 3, DL], BF)
            fl_flat = fl_d.rearrange("b l d -> (b l) d")
            nc.sync.dma_start(
                FLn[:, 0:2, :], fl_flat[: 2 * P].rearrange("(t p) d -> p t d", p=P)
            )
            nc.sync.dma_start(FLn[: NLB - 2 * P, 2, :], fl_flat[2 * P :, :])

            # fv0 first half ahead of wkc: its transposes are the first PE work
            # after FLT; wkc ahead of the packed small consts (GT needs it next)
            FVn0 = fvnp.tile([P, 8, DV], BF, tag="fvn")
            fvb0 = fv_d[0].rearrange("(t p) d -> p t d", p=P)
            nc.sync.dma_start(FVn0[:, 0:4, :], fvb0[:, 0:4, :])
            Wkc = lpt.tile([P, 4, DV], BF)
            nc.sync.dma_start(Wkc, wkc_d.rearrange("(ko p) m -> p ko m", p=P))
            nc.sync.dma_start(FVn0[:, 4:8, :], fvb0[:, 4:8, :])

            cpkb = cp.tile([P, 10], BF)
            nc.sync.dma_start(cpkb, cpkb_d[:, :])
            cpkf = cp.tile([P, 12], F32)
            nc.sync.dma_start(cpkf, cpkf_d[:, :])
            wct = cpkb[:, 0:4]
            wcvt = cpkb[:, 4:8]
            ccvt = cpkb[:1, 8:9]
            c2t = cpkf[:, 0:6]
            c2vt = cpkf[:, 6:12]
            WmE = cp.tile([97, OD], BF)
            nc.sync.dma_start(WmE, wme_d[:, :])
            bmr = cp.tile([1, OD], BF)
            nc.sync.dma_start(bmr, bmr_d[:, :])
            Wvc = lpt.tile([P, 4, DV], BF)
            nc.sync.dma_start(Wvc, wvc_d.rearrange("(ko p) m -> p ko m", p=P))

            # ---------------- language phase part 1: FLT, GT, c ----------------
            # FLT = fl_all^T  [512(D1 on p), 308]
            FLT = lpt.tile([P, 4, NLB], BF)
            for ko in range(4):
                ps = tp.tile([P, 384], BF, tag="tp")
                for i in range(3):
                    nc.tensor.transpose(
                        ps[:, i * P : (i + 1) * P],
                        FLn[:, i, ko * P : (ko + 1) * P],
                        iden,
                    )
                if ko % 2 == 0:
                    nc.vector.tensor_copy(FLT[:, ko, :], ps[:, :NLB])
                else:
                    nc.scalar.activation(FLT[:, ko, :], ps[:, :NLB], AF.Copy)

            def _emit_tg(FVn, tg):
                fvth = fvtp.tile([P, 6, 512], BF, tag="fvt")
                for ko in range(6):
                    ps = tp.tile([P, 512], BF, tag="tp")
                    for tt in range(4):
                        t = tg * 4 + tt
                        nc.tensor.transpose(
                            ps[:, tt * P : (tt + 1) * P],
                            FVn[:, t, ko * P : (ko + 1) * P],
                            iden,
                        )
                    if (ko + tg) % 2 == 0:
                        nc.vector.tensor_copy(fvth[:, ko, :], ps)
                    else:
                        nc.scalar.activation(fvth[:, ko, :], ps, AF.Copy)
                return fvth

            # fv0's first transpose group before GT: fv0-q0 lands before wkc
            tg00 = _emit_tg(FVn0, 0)

            # GT = g^T = (W_lk @ W_vk^T)^T @ fl^T + c2 : [768, 308]
            GT = lp.tile([P, 6, NLB], BF)
            for mv in range(6):
                ps = tp.tile([P, NLB], F32, tag="tp")
                for ko in range(4):
                    nc.tensor.matmul(
                        ps, Wkc[:, ko, mv * P : (mv + 1) * P], FLT[:, ko, :],
                        start=(ko == 0), stop=(ko == 3),
                    )
                if mv % 2 == 0:
                    nc.vector.tensor_scalar_add(GT[:, mv, :], ps, c2t[:, mv, None])
                else:
                    nc.scalar.activation(
                        GT[:, mv, :], ps, AF.Identity, bias=c2t[:, mv, None]
                    )

            # cbias[:, b] = (fk_l @ b_vk) * ISQD as a column per batch : [77, 4]
            psc = tp.tile([NL, 4], F32, tag="tp")
            for b in range(BL):
                ls = b * NL
                for ko in range(4):
                    nc.tensor.matmul(
                        psc[:, b : b + 1], FLT[:, ko, ls : ls + NL],
                        wct[:, ko : ko + 1],
                        start=(ko == 0), stop=(ko == 3),
                    )
            cbias = cp.tile([NL, 4], F32)
            nc.scalar.mul(cbias, psc, ISQD)

            # persistent Y ping-pong pair: Y = m_small @ W_m  [77, 768] per
            # batch, extended with row 77 = b_m (written once; rows 0..76 are
            # rewritten every batch). The finals contract E_ext^T @ Y_ext.
            YS = []
            for _i in range(2):
                _y = lp.tile([97, OD], BF, tag=f"YS{_i}")
                nc.gpsimd.memset(_y[64:96, :], 0.0)
                nc.vector.tensor_copy(_y[96:97, :], bmr)
                YS.append(_y)

            # ---------------- pipelined per-batch machinery ----------------
            finals_q = []

            def _pop(n=1):
                for _ in range(n):
                    if finals_q:
                        finals_q.pop(0)()

            def _issue_load(nb):
                FVn = fvnp.tile([P, 8, DV], BF, tag="fvn")
                fvb = fv_d[nb].rearrange("(t p) d -> p t d", p=P)
                for q in range(2):
                    nc.sync.dma_start(
                        FVn[:, 4 * q : 4 * q + 4, :], fvb[:, 4 * q : 4 * q + 4, :]
                    )
                return FVn

            def _emit_araw_E(nb, FVTh):
                nls = nb * NL
                # a_raw = g @ fv^T -> 2x 1-bank psum [77, 512] halves
                E = ebp.tile([P, NV], BF, tag="E")
                if nb < 2:
                    # rows 77..95 are read (x0) by the finals' [0:97) lhsT;
                    # zero them once per ring buffer (never written again)
                    nc.gpsimd.memset(E[64:96, :], 0.0)
                s1p = smp.tile([NL, 2], F32, tag="s1p")
                for nv in range(2):
                    sl = tp.tile([NL, 512], F32, tag="tp")
                    for ko in range(6):
                        nc.tensor.matmul(
                            sl, GT[:, ko, nls : nls + NL],
                            FVTh[nv][:, ko, :],
                            start=(ko == 0), stop=(ko == 5),
                        )
                    nc.scalar.activation(
                        E[:NL, nv * 512 : (nv + 1) * 512], sl,
                        AF.Exp, scale=ISQD, bias=cbias[:, nb : nb + 1],
                        accum_out=s1p[:, nv, None],
                    )
                    if nv == 0:
                        _pop()
                s1 = smp.tile([NL, 1], F32, tag="s1")
                nc.vector.reduce_sum(s1, s1p, axis=mybir.AxisListType.X)
                ivs1 = smp.tile([NL, 1], F32, tag="ivs1")
                nc.vector.reciprocal(ivs1, s1)
                return (E, ivs1)

            # ---------------- batch 0 head ----------------
            FVTh0 = [tg00, _emit_tg(FVn0, 1)]
            vstate = {0: [FVn0, FVTh0, None]}
            vstate[0][2] = _emit_araw_E(0, FVTh0)

            # language phase part 2 (needed only from m_small onward): runs on
            # PE while the Activation engine computes batch 0's exp.
            FWVT = lp.tile([P, 6, NLB], BF)
            for mv in range(6):
                ps = tp.tile([P, NLB], F32, tag="tp")
                for ko in range(4):
                    nc.tensor.matmul(
                        ps, Wvc[:, ko, mv * P : (mv + 1) * P], FLT[:, ko, :],
                        start=(ko == 0), stop=(ko == 3),
                    )
                if mv % 2 == 0:
                    nc.vector.tensor_scalar_add(FWVT[:, mv, :], ps, c2vt[:, mv, None])
                else:
                    nc.scalar.activation(
                        FWVT[:, mv, :], ps, AF.Identity, bias=c2vt[:, mv, None]
                    )

            # cv as a row: cvr[0, b*77+l] = (fv_l @ b_vv + b_lv@b_vv)[l]
            pscv = tp.tile([1, NLB], F32, tag="tp")
            for ko in range(4):
                nc.tensor.matmul(
                    pscv, wcvt[:, ko : ko + 1], FLT[:, ko, :],
                    start=(ko == 0), stop=False,
                )
            nc.tensor.matmul(
                pscv, ccvt[:1, :1], onesr[:1, :NLB], start=False, stop=True
            )
            cvr = cp.tile([1, NLB], BF)
            nc.vector.tensor_copy(cvr, pscv)
            lph_tmp.__exit__(None, None, None)

            # ---------------- per-batch vision phase ----------------
            for b in range(BL):
                ls = b * NL

                FVn, FVTh, pre = vstate.pop(b)
                E, ivs1 = pre

                if b + 1 < BL:
                    FVn1 = _issue_load(b + 1)
                    vstate[b + 1] = [FVn1, None, None]
                _pop()

                # E^T blocks + s2 (column sums of E)
                ET = smp.tile([P, 8, NL], BF, tag="ET")
                s2 = smp.tile([P, 8], F32, tag="s2")
                for tg in range(2):
                    ps = tp.tile([P, 512], BF, tag="tp")
                    for tt in range(4):
                        t = tg * 4 + tt
                        nc.tensor.transpose(
                            ps[:, tt * P : (tt + 1) * P],
                            E[:, t * P : (t + 1) * P],
                            iden,
                        )
                    psv = ps.rearrange("p (four c) -> p four c", four=4)[:, :, :NL]
                    if tg == 0:
                        nc.scalar.activation(ET[:, 0:4, :], psv, AF.Copy)
                    else:
                        nc.vector.tensor_copy(ET[:, 4:8, :], psv)
                    nc.vector.reduce_sum(
                        s2[:, tg * 4 : (tg + 1) * 4],
                        ET[:, tg * 4 : (tg + 1) * 4, :],
                        axis=mybir.AxisListType.X,
                    )
                ivs2 = smp.tile([P, 8], F32, tag="ivs2")
                nc.vector.reciprocal(ivs2, s2)
                _pop()

                # s2 row into E row 77 (pre-multiplies b_m by s2 in the finals)
                for nv in range(2):
                    ps2 = tp.tile([1, 512], F32, tag="tp")
                    nc.tensor.matmul(
                        ps2, onesc[:NL, :], E[:NL, nv * 512 : (nv + 1) * 512],
                        start=True, stop=True,
                    )
                    if nv == 0:
                        nc.vector.tensor_copy(E[96:97, :512], ps2)
                    else:
                        nc.scalar.activation(E[96:97, 512:], ps2, AF.Copy)
                _pop()

                # h1T_un = (E @ fv)^T directly: [768, 77] in 6 chunk-rows of a
                # single 1-bank psum tile (contraction over v on partitions)
                psh = tp.tile([P, 6, NL], F32, tag="tp")
                for ko in range(6):
                    sl = psh[:, ko, :]
                    for t in range(8):
                        nc.tensor.matmul(
                            sl, FVn[:, t, ko * P : (ko + 1) * P], ET[:, t, :],
                            start=(t == 0), stop=(t == 7),
                        )
                    if ko == 1:
                        _pop()
                H1T = smp.tile([P, 6, NL], BF, tag="H1T")
                nc.scalar.activation(H1T, psh, AF.Copy)

                # fv^T transposes for the next batch fill the H1T wait
                if b + 1 < BL:
                    vstate[b + 1][1] = [_emit_tg(vstate[b + 1][0], 0)]
                _pop()

                # MT = m_small^T (1/s1 applied on copy-back); row 77 = cv
                MT = smp.tile([97, NL], BF, tag="MT")
                if b < 2:
                    # rows 77..95 are read (x0) by the Y matmul's [0:97) lhsT
                    nc.gpsimd.memset(MT[64:96, :], 0.0)
                psm = tp.tile([NL, NL], F32, tag="tp")
                for ko in range(6):
                    nc.tensor.matmul(
                        psm, H1T[:, ko, :], FWVT[:, ko, ls : ls + NL],
                        start=(ko == 0), stop=(ko == 5),
                    )
                nc.scalar.activation(MT[:NL, :], psm, AF.Identity, scale=ivs1)
                nc.vector.tensor_copy(MT[96:97, :], cvr[:, ls : ls + NL])

                # second transpose group of next batch fills the MT wait
                if b + 1 < BL:
                    vstate[b + 1][1].append(_emit_tg(vstate[b + 1][0], 1))
                _pop()

                # Y = m_small @ W_m (cv x colsum(W_m) enters via MT/WmE row 77)
                Y = YS[b % 2]
                psy = accp.tile([NL, OD], F32, tag="acc")
                for c0, cw in ((0, 512), (512, 256)):
                    nc.tensor.matmul(
                        psy[:, c0 : c0 + cw], MT, WmE[:, c0 : c0 + cw],
                        start=True, stop=True,
                    )
                nc.vector.tensor_copy(Y[:NL, :], psy)

                # enqueue this batch's 8 finals as single-tile chunks. The last
                # batch drains with no other PE work to hide PSUM-ring waits,
                # so it uses 1-bank psum pieces from the deeper tp ring.
                deep = b == BL - 1
                def _emit_final_t(b=b, E=E, Y=Y, ivs2=ivs2, deep=deep, t=0):
                    OT = outp.tile([P, OD], BF, tag="OT")
                    if deep:
                        for k, (c0, cw) in enumerate(((0, 512), (512, 256))):
                            pso = tp.tile([P, cw], F32, tag="tp")
                            nc.tensor.matmul(
                                pso, E[:97, t * P : (t + 1) * P],
                                Y[:, c0 : c0 + cw], start=True, stop=True,
                            )
                            if (t + k) % 2 == 0:
                                nc.vector.tensor_scalar_mul(
                                    OT[:, c0 : c0 + cw], pso, ivs2[:, t, None]
                                )
                            else:
                                nc.scalar.activation(
                                    OT[:, c0 : c0 + cw], pso,
                                    AF.Identity, scale=ivs2[:, t, None],
                                )
                    else:
                        pso = accp.tile([P, OD], F32, tag="acc")
                        for c0, cw in ((0, 512), (512, 256)):
                            nc.tensor.matmul(
                                pso[:, c0 : c0 + cw], E[:97, t * P : (t + 1) * P],
                                Y[:, c0 : c0 + cw], start=True, stop=True,
                            )
                        if t % 2 == 0:
                            nc.vector.tensor_scalar_mul(OT, pso, ivs2[:, t, None])
                        else:
                            nc.scalar.activation(
                                OT, pso, AF.Identity, scale=ivs2[:, t, None]
                            )
                    nc.sync.dma_start(out_d[b, t * P : (t + 1) * P, :], OT)
                import functools as _ft
                for t in range(8):
                    finals_q.append(_ft.partial(_emit_final_t, t=t))
                if b == 0:
                    _pop()  # get the first store to the idle DMA engines early

                # next batch's a_raw + exp (Act overlaps the pops below)
                if b + 1 in vstate:
                    vstate[b + 1][2] = _emit_araw_E(b + 1, vstate[b + 1][1])
                _pop(2)

            _pop(len(finals_q))

    nc.compile()
    return nc


_NC_CACHE = None
_last_in_maps = None


def kernel(**inputs) -> np.ndarray:
    bf = ml_dtypes.bfloat16
    f32 = np.float32
    fv = np.asarray(inputs["fv"], f32)
    fl = np.asarray(inputs["fl"], f32)
    W_vk = np.asarray(inputs["W_vk"], f32)
    b_vk = np.asarray(inputs["b_vk"], f32)
    W_vv = np.asarray(inputs["W_vv"], f32)
    b_vv = np.asarray(inputs["b_vv"], f32)
    W_lk = np.asarray(inputs["W_lk"], f32)
    b_lk = np.asarray(inputs["b_lk"], f32)
    W_lv = np.asarray(inputs["W_lv"], f32)
    b_lv = np.asarray(inputs["b_lv"], f32)
    W_m = np.asarray(inputs["W_m"], f32)
    b_m = np.asarray(inputs["b_m"], f32)

    wct_pk = (W_lk @ b_vk).reshape(4, P).T
    wcvt_pk = (W_lv @ b_vv).reshape(4, P).T
    ccv = float(b_lv @ b_vv)
    consts = {
        "wkc": np.ascontiguousarray(W_lk @ W_vk.T).astype(bf),
        "wvc": np.ascontiguousarray(W_lv @ W_vv.T).astype(bf),
        "cpkb": np.concatenate(
            [wct_pk, wcvt_pk, np.full((P, 2), ccv, f32)], axis=1
        ).astype(bf),
        "cpkf": np.concatenate(
            [(W_vk @ b_lk).reshape(6, P).T, (W_vv @ b_lv).reshape(6, P).T], axis=1
        ).astype(f32),
        "wme": np.concatenate(
            [W_m, np.zeros((19, OD), f32), W_m.sum(0, keepdims=True)], axis=0
        ).astype(bf),
        "bmr": b_m[None, :].astype(bf),
        "iden": np.eye(P, dtype=bf),
    }
    fvb = fv.astype(bf)
    flb = fl.astype(bf)
    in_maps = []
    for c in range(NCORES):
        m = dict(consts)
        m["fv"] = np.ascontiguousarray(fvb[c * BL : (c + 1) * BL])
        m["fl"] = np.ascontiguousarray(flb[c * BL : (c + 1) * BL])
        in_maps.append(m)

    global _last_in_maps
    _last_in_maps = in_maps
    nc = _get_nc()
    res = run_bass_kernel_spmd(nc, in_maps, core_ids=list(range(NCORES)))
    out = np.concatenate([res.results[c]["out"] for c in range(NCORES)], axis=0)
    return np.ascontiguousarray(out, dtype=np.float32)


def _get_nc():
    global _NC_CACHE
    if _NC_CACHE is None:
        _NC_CACHE = _build()
    return _NC_CACHE

